# revision 50
# baseline (speedup 1.0000x reference)
"""Trainium2 Bass kernel for nn_DiffOp_8813272892073 (2-layer GNN message passing).

Strategy
--------
All per-edge GEMMs collapse algebraically: concat([x_src, x_tgt]) @ W.T splits
into x_src @ A + x_tgt @ B, so the scatter-mean of messages reduces to
  d(v) = invc(v) * [ (ceff*x)@Wself + (cnt_int*x)@C_xii (+ cnt_b/cnt_c terms in L2)
                     + segsum_{e->v}( g[src_e] ) + A(v) ]
where g[n] = x[n] @ C_S is a host- (L1) or device- (L2) transformed gather table,
and A(v) holds the tiny boundary/control contributions + biases (host-computed).

Device work per core (nodes sharded contiguously, edges bucketed by target):
  - dma_gather of g[src] rows for this core's edges (sorted by target,
    128-edge chunks aligned to 128-target output tiles; indices are int16 so
    the table is split into lo/hi halves with separate chunks)
  - segment-sum via one-hot indicator matmuls accumulating in PSUM
    (indicators built on DVE via is_equal and on the otherwise-idle ACT
    engine via an exact Abs/Relu construction, to balance engine load)
  - node-update GEMMs with stationary weights into the same PSUM group
  - count/inv-count scale rows replicated across partitions on the fly with
    K=1 outer-product matmuls into PSUM (read directly by DVE)
  - an AllGather of the layer-2 gather table between the two layers.
Everything is computed in a transposed layout [feat(partition), node(free)];
the host transposes the final output back.
"""
import numpy as np
import ml_dtypes

import concourse.bass as bass
import concourse.bacc as bacc
import concourse.tile as tile
import concourse.mybir as mybir
from concourse import bass_utils, library_config

BF16 = ml_dtypes.bfloat16

# problem constants (hardcoded per contract)
N_INT = 50000
IN_DIM = 128
D = 128
HL = 32768                       # lo-table rows (int16 index limit)
N_ACT = 4                        # indicator chunks per tile built on ACT engine


class Cfg:
    def __init__(self, n_cores=8, npc=6250, lcpt=6, hcpt=4, g=7, d=128, hl=HL):
        self.n_cores = n_cores
        self.npc = npc                       # nodes per core (unpadded)
        self.tiles = (npc + 127) // 128      # 128-target tiles per core
        assert self.tiles % g == 0, (self.tiles, g)
        self.ng = self.tiles // g            # gather groups
        self.g = g                           # tiles per gather group
        self.lcpt = lcpt                     # lo-table chunks per tile
        self.hcpt = hcpt                     # hi-table chunks per tile
        self.cpt = lcpt + hcpt
        self.npad = self.tiles * 128         # padded nodes per core
        self.npf = self.npad * n_cores       # padded total nodes
        self.hl = hl if hl < self.npf else self.npf // 2  # lo/hi table split
        self.d = d

    def block_of(self, g_in_group, c):
        """gbuf chunk-block index for tile-in-group g, tile-chunk c."""
        if c < self.lcpt:
            return g_in_group * self.lcpt + c
        return self.g * self.lcpt + g_in_group * self.hcpt + (c - self.lcpt)


# ---------------------------------------------------------------- device build

def build_nc(cfg: Cfg, repeat: int = 1, no_collective: bool = False):
    nc = bacc.Bacc("TRN2", target_bir_lowering=False, debug=False,
                   num_devices=cfg.n_cores)
    dt = mybir.dt
    T, G, NG, NPAD, NPF, d = cfg.tiles, cfg.g, cfg.ng, cfg.npad, cfg.npf, cfg.d
    LCPT, HCPT, CPT = cfg.lcpt, cfg.hcpt, cfg.cpt
    NLO = G * LCPT * 128                    # lo idxs per group
    NHI = G * HCPT * 128
    ts = bass.ts

    xg = nc.dram_tensor("xg", [NPF, d], dt.bfloat16, kind="ExternalInput")
    xt = nc.dram_tensor("xt", [NG, d, G * 2 * d], dt.bfloat16, kind="ExternalInput")
    a1 = nc.dram_tensor("a1", [NG, d, G * d], dt.bfloat16, kind="ExternalInput")
    a2 = nc.dram_tensor("a2", [NG, d, G * d], dt.bfloat16, kind="ExternalInput")
    svc = nc.dram_tensor("svc", [1, 4 * NPAD], dt.bfloat16, kind="ExternalInput")
    svi = nc.dram_tensor("svi", [1, NPAD], dt.float32, kind="ExternalInput")
    wmat = nc.dram_tensor("wmat", [d, 7 * d], dt.bfloat16, kind="ExternalInput")
    iotp = nc.dram_tensor("iotp", [d, d], dt.bfloat16, kind="ExternalInput")
    iotn = nc.dram_tensor("iotn", [d, d], dt.bfloat16, kind="ExternalInput")
    iot4n = nc.dram_tensor("iot4n", [d, 4 * d], dt.bfloat16, kind="ExternalInput")
    pcol = nc.dram_tensor("pcol", [d, 1], dt.bfloat16, kind="ExternalInput")
    sidx = nc.dram_tensor("sidx", [NG, d, (NLO + NHI) // 16], dt.int16,
                          kind="ExternalInput")
    rel = nc.dram_tensor("rel", [NG, d, G * CPT], dt.bfloat16, kind="ExternalInput")
    dxT = nc.dram_tensor("dxT", [d, NPAD], dt.float32, kind="ExternalOutput")

    with tile.TileContext(nc) as tc:
        with (
            tc.tile_pool(name="const", bufs=1) as const,
            tc.tile_pool(name="stream", bufs=3) as stream,
            tc.tile_pool(name="gpool", bufs=2) as gpool,
            tc.tile_pool(name="indp", bufs=12) as indp,
            tc.tile_pool(name="psum", bufs=2, space="PSUM") as psum,
            tc.tile_pool(name="dram", bufs=1, space="DRAM") as dram,
        ):
            nc.gpsimd.load_library(library_config.mlp)

            wmat_sb = const.tile([d, 7 * d], dt.bfloat16)
            nc.sync.dma_start(out=wmat_sb[:], in_=wmat[:])
            iotp_sb = const.tile([d, d], dt.bfloat16)
            nc.sync.dma_start(out=iotp_sb[:], in_=iotp[:])
            iot4n_sb = const.tile([d, 4 * d], dt.bfloat16)
            nc.sync.dma_start(out=iot4n_sb[:], in_=iot4n[:])
            pcol_sb = const.tile([d, 1], dt.bfloat16)
            nc.sync.dma_start(out=pcol_sb[:], in_=pcol[:])
            ones_c = const.tile([1, d], dt.bfloat16)
            nc.vector.memset(ones_c[:], 1.0)
            ones_f = const.tile([1, d], dt.float32)
            nc.vector.memset(ones_f[:], 1.0)
            d1T = const.tile([d, NPAD], dt.bfloat16)

            # bf16 identity (exact): ident[p, f] = (p == f)
            ident_sb = const.tile([d, d], dt.bfloat16)
            nc.vector.tensor_tensor(out=ident_sb[:],
                                    in0=pcol_sb[:, 0:1].to_broadcast([d, d]),
                                    in1=iotp_sb[:], op=mybir.AluOpType.is_equal)

            # SBUF-resident per-node scale tiles, replicated across partitions
            # via K=1 outer-product matmuls (PSUM) + evacuation copies.
            invc_rep = const.tile([d, NPAD], dt.float32)
            cnts_rep = const.tile([d, 4 * NPAD], dt.bfloat16)
            svi_st, svi_free = tc.tile([1, NPAD], dt.float32, name="svi_st")
            nc.sync.dma_start(out=svi_st[:], in_=svi[:])
            offi = 0
            while offi < NPAD:
                w = min(512, NPAD - offi)
                pbld = psum.tile([d, 512], dt.float32, name=f"pbi_{offi}",
                                 tag="pbld")
                nc.tensor.matmul(out=pbld[:, 0:w], lhsT=ones_f[0:1, :],
                                 rhs=svi_st[0:1, offi:offi + w],
                                 start=True, stop=True)
                nc.scalar.activation(invc_rep[:, offi:offi + w], pbld[:, 0:w],
                                     mybir.ActivationFunctionType.Copy)
                offi += w
            svi_free()
            # svc is tile-major: tile t slot k at cols [t*512+k*128, ...).
            off = 0
            piece = 0
            while off < 4 * NPAD:
                sz = min(6144, 4 * NPAD - off)
                svc_st, svc_free = tc.tile([1, 6144], dt.bfloat16,
                                           name=f"svcst{piece}")
                nc.sync.dma_start(out=svc_st[:, 0:sz], in_=svc[0:1, off:off + sz])
                for b in range(sz // 512):
                    pbld = psum.tile([d, 512], dt.float32,
                                     name=f"pbc{piece}_{b}", tag="pbld")
                    nc.tensor.matmul(out=pbld[:], lhsT=ones_c[0:1, :],
                                     rhs=svc_st[0:1, ts(b, 512)],
                                     start=True, stop=True)
                    nc.scalar.activation(
                        cnts_rep[:, off + b * 512:off + (b + 1) * 512],
                        pbld[:], mybir.ActivationFunctionType.Copy)
                svc_free()
                off += sz
                piece += 1

            g2loc = dram.tile([NPAD, d], dt.bfloat16)
            g2full = dram.tile([NPF, d], dt.bfloat16)

            def gather_group(gi, table, phase):
                idx_t = stream.tile([d, (NLO + NHI) // 16], dt.int16,
                                    name=f"idx{phase}_{gi}", tag="idx", bufs=4)
                nc.sync.dma_start(out=idx_t[:], in_=sidx[gi])
                gbuf = gpool.tile([d, G * CPT * d], dt.bfloat16,
                                  name=f"gbuf{phase}_{gi}", tag="gbuf")
                nc.gpsimd.dma_gather(
                    gbuf[:, 0:NLO].rearrange("p (c e) -> p c e", e=d),
                    table[0:cfg.hl, :],
                    idx_t[:, 0:NLO // 16],
                    NLO, NLO, d, single_packet=False,
                )
                nc.gpsimd.dma_gather(
                    gbuf[:, NLO:].rearrange("p (c e) -> p c e", e=d),
                    table[cfg.hl:NPF, :],
                    idx_t[:, NLO // 16:],
                    NHI, NHI, d, single_packet=False,
                )
                return gbuf

            def seg_mms(uid, gbuf, rel_t, g_in_group, psumN):
                base = g_in_group * CPT
                for c0 in range(0, CPT, 4):
                    w = min(4, CPT - c0)
                    ind4 = indp.tile([d, 4 * d], dt.bfloat16,
                                     name=f"ind{uid}_{c0}", tag="ind4", bufs=10)
                    nc.vector.tensor_tensor(
                        out=ind4[:, 0:w * d].rearrange("p (c e) -> p c e", e=d),
                        in0=rel_t[:, base + c0:base + c0 + w, None
                                  ].to_broadcast([d, w, d]),
                        in1=iot4n_sb[:, 0:w * d].rearrange("p (c e) -> p c e", e=d),
                        op=mybir.AluOpType.is_equal)
                    for c in range(c0, c0 + w):
                        blk = cfg.block_of(g_in_group, c)
                        nc.tensor.matmul(
                            out=psumN[:],
                            lhsT=gbuf[:, blk * d:(blk + 1) * d],
                            rhs=ind4[:, (c - c0) * d:(c - c0 + 1) * d],
                            start=(c == 0), stop=False,
                        )

            def l1_group(gi, rep):
                gbuf = gather_group(gi, xg, 1000 + rep)
                rel_t = stream.tile([d, G * CPT], dt.bfloat16,
                                    name=f"rl1_{rep}_{gi}", tag="rel", bufs=4)
                nc.sync.dma_start(out=rel_t[:], in_=rel[gi])
                xt_t = stream.tile([d, G * 2 * d], dt.bfloat16,
                                   name=f"xt{rep}_{gi}", tag="xt")
                nc.sync.dma_start(out=xt_t[:], in_=xt[gi])
                a1_t = stream.tile([d, G * d], dt.bfloat16,
                                   name=f"a1t{rep}_{gi}", tag="at")
                nc.sync.dma_start(out=a1_t[:], in_=a1[gi])
                g2st = stream.tile([d, G * d], dt.bfloat16,
                                   name=f"g2s{rep}_{gi}", tag="g2s")
                for g in range(G):
                    ti = gi * G + g
                    uid = f"1_{rep}_{ti}"
                    psumN = psum.tile([d, d], dt.float32, name=f"p{uid}",
                                      tag="psumN", bufs=3)
                    seg_mms(uid, gbuf, rel_t, g, psumN)
                    nc.tensor.matmul(out=psumN[:], lhsT=wmat_sb[:, 0:d],
                                     rhs=xt_t[:, ts(2 * g, d)],
                                     start=False, stop=False)
                    nc.tensor.matmul(out=psumN[:], lhsT=wmat_sb[:, d:2 * d],
                                     rhs=xt_t[:, ts(2 * g + 1, d)],
                                     start=False, stop=False)
                    nc.tensor.matmul(out=psumN[:], lhsT=ident_sb[:],
                                     rhs=a1_t[:, ts(g, d)],
                                     start=False, stop=True)
                    nc.vector.tensor_tensor(out=d1T[:, ts(ti, d)], in0=psumN[:],
                                            in1=invc_rep[:, ts(ti, d)],
                                            op=mybir.AluOpType.mult)
                    # layer-2 gather table rows: g2 = d1 @ C2S, row-major
                    psumG = psum.tile([d, d], dt.float32, name=f"pg{uid}",
                                      tag="psumG", bufs=3)
                    nc.tensor.matmul(out=psumG[:], lhsT=d1T[:, ts(ti, d)],
                                     rhs=wmat_sb[:, 2 * d:3 * d],
                                     start=True, stop=True)
                    nc.scalar.activation(g2st[:, ts(g, d)], psumG[:],
                                         mybir.ActivationFunctionType.Copy)
                nc.sync.dma_start(
                    out=g2loc[gi * G * d:(gi + 1) * G * d, :].rearrange(
                        "(g t) f -> t g f", g=G),
                    in_=g2st[:].rearrange("p (g f) -> p g f", f=d))

            def l2_group(gi, rep):
                gbuf = gather_group(gi, g2full, 2000 + rep)
                rel_t = stream.tile([d, G * CPT], dt.bfloat16,
                                    name=f"rl2_{rep}_{gi}", tag="rel", bufs=4)
                nc.sync.dma_start(out=rel_t[:], in_=rel[gi])
                a2_t = stream.tile([d, G * d], dt.bfloat16,
                                   name=f"a2t{rep}_{gi}", tag="at")
                nc.sync.dma_start(out=a2_t[:], in_=a2[gi])
                dxst = stream.tile([d, G * d], dt.float32,
                                   name=f"dxs{rep}_{gi}", tag="dxs")
                for g in range(G):
                    ti = gi * G + g
                    uid = f"2_{rep}_{ti}"
                    psumN = psum.tile([d, d], dt.float32, name=f"p{uid}",
                                      tag="psumN", bufs=3)
                    seg_mms(uid, gbuf, rel_t, g, psumN)
                    sc4 = indp.tile([d, 4 * d], dt.bfloat16, name=f"sc{uid}",
                                    tag="sc4")
                    nc.vector.tensor_tensor(
                        out=sc4[:].rearrange("p (c e) -> p c e", e=d),
                        in0=d1T[:, ti * d:(ti + 1) * d, None].rearrange(
                            "p e one -> p one e").to_broadcast([d, 4, d]),
                        in1=cnts_rep[:, ts(ti, 4 * d)].rearrange(
                            "p (c e) -> p c e", e=d),
                        op=mybir.AluOpType.mult)
                    for k in range(4):
                        nc.tensor.matmul(out=psumN[:],
                                         lhsT=wmat_sb[:, ts(3 + k, d)],
                                         rhs=sc4[:, ts(k, d)],
                                         start=False, stop=False)
                    nc.tensor.matmul(out=psumN[:], lhsT=ident_sb[:],
                                     rhs=a2_t[:, ts(g, d)],
                                     start=False, stop=True)
                    nc.vector.tensor_tensor(out=dxst[:, ts(g, d)], in0=psumN[:],
                                            in1=invc_rep[:, ts(ti, d)],
                                            op=mybir.AluOpType.mult)
                nc.sync.dma_start(out=dxT[:, ts(gi, G * d)], in_=dxst[:])

            GP = G * 128                        # rows per group piece
            for rep in range(repeat):
                for gi in range(NG):
                    l1_group(gi, rep)
                    # group-major global layout: piece gi holds all cores'
                    # rows for group gi, concatenated by core.
                    if no_collective:
                        nc.sync.dma_start(
                            out=g2full[gi * cfg.n_cores * GP:
                                       gi * cfg.n_cores * GP + GP, :],
                            in_=g2loc[gi * GP:(gi + 1) * GP, :])
                    else:
                        nc.gpsimd.collective_compute(
                            "AllGather",
                            mybir.AluOpType.bypass,
                            replica_groups=[list(range(cfg.n_cores))],
                            ins=[g2loc[gi * GP:(gi + 1) * GP, :].opt()],
                            outs=[g2full[gi * cfg.n_cores * GP:
                                         (gi + 1) * cfg.n_cores * GP, :].opt()],
                        )
                for gi in range(NG):
                    l2_group(gi, rep)

    nc.compile()
    return nc


# ------------------------------------------------------------------- host prep

def _linear_mats(p, in_dim, ctrl_dim):
    W = lambda n: np.asarray(p[n]['W'], np.float64)
    b = lambda n: np.asarray(p[n]['b'], np.float64)
    M = W('int_msg').T
    return dict(
        M=M,
        C_S=W('ii')[:, :in_dim].T @ M,
        C_xii=W('ii')[:, in_dim:].T @ M,
        C_xb=W('bi')[:, in_dim:].T @ M,
        C_xc=W('ci')[:, ctrl_dim:].T @ M,
        A_b=W('bi')[:, :in_dim].T,
        A_c=W('ci')[:, :ctrl_dim].T,
        Wself=W('int_self').T,
        b_ii=b('ii'), b_bi=b('bi'), b_ci=b('ci'),
        bias=b('int_self') + b('int_msg'),
    )


def _wrap_idx(stream_arr):
    """[K] int -> [128, K//16] int16 (idx k at [k%16, k//16], replicated x8)."""
    w16 = stream_arr.reshape(-1, 16).T.astype(np.int16)    # [16, K//16]
    return np.tile(w16, (8, 1))


def host_prep(t, x_int_t, X_bound, U, edge_index_int, edge_index_bound,
              edge_index_ctrl, timestamps, params1, params2, cfg: Cfg):
    N = cfg.npc * cfg.n_cores
    d = cfg.d
    x = np.asarray(x_int_t, np.float64)
    if x.ndim == 3:
        x = x[0]
    ts_arr = np.asarray(timestamps, np.float64)
    tval = float(np.asarray(t).reshape(-1)[0])
    Xb = np.asarray(X_bound, np.float64)
    Uu = np.asarray(U, np.float64)

    last = len(ts_arr) - 1
    k = int(np.clip(np.searchsorted(ts_arr, tval, side='right') - 1, 0, last - 1))
    s = (tval - ts_arr[k]) / (ts_arr[k + 1] - ts_arr[k])
    xb = (1 - s) * Xb[k] + s * Xb[k + 1]
    ut = (1 - s) * Uu[k] + s * Uu[k + 1]
    if tval > ts_arr[last]:
        xb, ut = Xb[last], Uu[last]

    src = np.asarray(edge_index_int[0], np.int64)
    tgt = np.asarray(edge_index_int[1], np.int64)
    b_tgt = np.asarray(edge_index_bound[1], np.int64)
    c_tgt = np.asarray(edge_index_ctrl[1], np.int64)

    cnt_int = np.bincount(tgt, minlength=N).astype(np.float64)
    cnt_b = np.bincount(b_tgt, minlength=N).astype(np.float64)
    cnt_c = np.bincount(c_tgt, minlength=N).astype(np.float64)
    c_eff = np.maximum(cnt_int + cnt_b + cnt_c, 1.0)
    invc = 1.0 / c_eff

    in_dim = x.shape[1]
    ctrl_dim = ut.shape[1]
    m1 = _linear_mats(params1, in_dim, ctrl_dim)
    m2 = _linear_mats(params2, d, d)

    def small_updates(p, xb_a, u_a, idim, cdim):
        W = lambda n: np.asarray(p[n]['W'], np.float64)
        b = lambda n: np.asarray(p[n]['b'], np.float64)
        sb = xb_a @ (W('bb')[:, :idim] + W('bb')[:, idim:]).T + b('bb')
        b_out = xb_a @ W('b_self').T + b('b_self') + sb @ W('b_msg').T + b('b_msg')
        sc = u_a @ (W('cc')[:, :cdim] + W('cc')[:, cdim:]).T + b('cc')
        u_out = u_a @ W('c_self').T + b('c_self') + sc @ W('c_msg').T + b('c_msg')
        return b_out, u_out

    b1, u1 = small_updates(params1, xb, ut, in_dim, ctrl_dim)

    def host_A(m, xb_a, u_a, x_a, layer1):
        A = np.zeros((N, d))
        rows_b = (xb_a @ m['A_b'] + m['b_bi']) @ m['M']
        if layer1:
            rows_b = rows_b + x_a[b_tgt] @ m['C_xb']
        np.add.at(A, b_tgt, rows_b)
        rows_c = (u_a @ m['A_c'] + m['b_ci']) @ m['M']
        if layer1:
            rows_c = rows_c + x_a[c_tgt] @ m['C_xc']
        np.add.at(A, c_tgt, rows_c)
        A += cnt_int[:, None] * (m['b_ii'] @ m['M'])[None, :]
        A += c_eff[:, None] * m['bias'][None, :]
        return A

    A1 = host_A(m1, xb, ut, x, True)
    A2 = host_A(m2, b1, u1, None, False)

    g1 = x @ m1['C_S']                             # layer-1 gather table

    # ---- edge bucketing: (core, tile, lo/hi) chunks of 128
    core_of = tgt // cfg.npc
    loc = tgt % cfg.npc
    tile_of = loc // 128
    rel_of = loc % 128
    s_core_of = src // cfg.npc
    s_loc = src % cfg.npc
    s_tile_of = s_loc // 128
    src_pad = ((s_tile_of // cfg.g) * (cfg.n_cores * cfg.g) +
               s_core_of * cfg.g + (s_tile_of % cfg.g)) * 128 + (s_loc % 128)
    is_hi = (src_pad >= cfg.hl).astype(np.int64)

    order = np.lexsort((is_hi, tile_of, core_of))
    s_src = src_pad[order]
    s_rel = rel_of[order]
    s_core = core_of[order]
    s_tile = tile_of[order]
    s_hi = is_hi[order]

    bucket = (s_core * cfg.tiles + s_tile) * 2 + s_hi
    nb = cfg.n_cores * cfg.tiles * 2
    start_of = np.zeros(nb + 1, np.int64)
    np.add.at(start_of, bucket + 1, 1)
    counts = start_of[1:].copy()
    start_of = np.cumsum(start_of)
    rank = np.arange(len(order)) - start_of[bucket]

    lo_counts = counts[0::2].reshape(cfg.n_cores, cfg.tiles)
    hi_counts = counts[1::2].reshape(cfg.n_cores, cfg.tiles)
    need_l = int(np.ceil(lo_counts.max() / 128))
    need_h = int(np.ceil(hi_counts.max() / 128))
    assert need_l <= cfg.lcpt and need_h <= cfg.hcpt, \
        f"need lcpt>={need_l}, hcpt>={need_h}"

    NLO = cfg.g * cfg.lcpt * 128
    NHI = cfg.g * cfg.hcpt * 128
    idx_stream = np.zeros((cfg.n_cores, cfg.ng, NLO + NHI), np.int64)
    # rel stores NEGATED local target; pad = +1 (never matches -iota; Abs>=1)
    rel_all = np.full((cfg.n_cores, cfg.ng, 128, cfg.g * cfg.cpt), 1.0, np.float32)

    jj = rank // 128
    pp = rank % 128
    g_in = s_tile % cfg.g
    gidx = s_tile // cfg.g
    pos_lo = (g_in * cfg.lcpt + jj) * 128 + pp
    pos_hi = NLO + (g_in * cfg.hcpt + jj) * 128 + pp
    pos = np.where(s_hi == 1, pos_hi, pos_lo)
    val = np.where(s_hi == 1, s_src - cfg.hl, s_src)
    idx_stream[s_core, gidx, pos] = val
    chunk_col = np.where(s_hi == 1, cfg.lcpt + jj, jj)
    rel_all[s_core, gidx, pp, g_in * cfg.cpt + chunk_col] = -s_rel

    sidx_all = np.zeros((cfg.n_cores, cfg.ng, 128, (NLO + NHI) // 16), np.int16)
    for c in range(cfg.n_cores):
        for gg in range(cfg.ng):
            sidx_all[c, gg, :, :NLO // 16] = _wrap_idx(idx_stream[c, gg, :NLO])
            sidx_all[c, gg, :, NLO // 16:] = _wrap_idx(idx_stream[c, gg, NLO:])

    # ---- per-core dense inputs
    def padT(a2d, c):  # [npc, d] slice of core c -> [d, npad]
        sl = a2d[c * cfg.npc:(c + 1) * cfg.npc]
        out = np.zeros((cfg.npad, a2d.shape[1]), np.float32)
        out[:cfg.npc] = sl
        return np.ascontiguousarray(out.T)

    # gather-table rows in the same group-major padded layout
    xg_pad = np.zeros((cfg.npf, d), np.float32)
    node = np.arange(cfg.n_cores * cfg.npc)
    n_core = node // cfg.npc
    n_loc = node % cfg.npc
    n_tile = n_loc // 128
    n_pad = ((n_tile // cfg.g) * (cfg.n_cores * cfg.g) +
             n_core * cfg.g + (n_tile % cfg.g)) * 128 + (n_loc % 128)
    xg_pad[n_pad] = g1[node]

    iota_p = np.broadcast_to(np.arange(d, dtype=np.float32), (d, d))
    wm = np.concatenate([m1['Wself'], m1['C_xii'], m2['C_S'], m2['Wself'],
                         m2['C_xii'], m2['C_xb'], m2['C_xc']], axis=1)

    xc = c_eff[:, None] * x
    xi = cnt_int[:, None] * x

    def group_tiles(mT, width):
        out = np.zeros((cfg.ng, d, cfg.g * width), np.float32)
        for ti in range(cfg.tiles):
            gg, g = divmod(ti, cfg.g)
            out[gg, :, g * width:(g + 1) * width] = mT[:, ti * 128:ti * 128 + width]
        return out

    in_maps = []
    for c in range(cfg.n_cores):
        xcT = padT(xc, c)
        xiT = padT(xi, c)
        a1T = padT(A1, c)
        a2T = padT(A2, c)
        xt_g = np.zeros((cfg.ng, d, cfg.g * 2 * d), np.float32)
        for ti in range(cfg.tiles):
            gg, g = divmod(ti, cfg.g)
            sl = slice(ti * 128, (ti + 1) * 128)
            xt_g[gg, :, (2 * g) * d:(2 * g + 1) * d] = xcT[:, sl]
            xt_g[gg, :, (2 * g + 1) * d:(2 * g + 2) * d] = xiT[:, sl]
        a1_g = group_tiles(a1T, d)
        a2_g = group_tiles(a2T, d)

        vecs = [c_eff, cnt_int, cnt_b, cnt_c]
        svc_g = np.zeros((1, 4 * cfg.npad), np.float32)
        for ti in range(cfg.tiles):
            lo = ti * 128
            n_here = max(0, min(cfg.npc - lo, 128))
            for kk, v in enumerate(vecs):
                if n_here > 0:
                    svc_g[0, ti * 512 + kk * d:ti * 512 + kk * d + n_here] = \
                        v[c * cfg.npc + lo:c * cfg.npc + lo + n_here]
        svi_g = np.ones((1, cfg.npad), np.float32)
        svi_g[0, :cfg.npc] = invc[c * cfg.npc:(c + 1) * cfg.npc]

        in_maps.append({
            "xg": xg_pad.astype(BF16),
            "xt": xt_g.astype(BF16),
            "a1": a1_g.astype(BF16),
            "a2": a2_g.astype(BF16),
            "svc": svc_g.astype(BF16),
            "svi": np.ascontiguousarray(svi_g, dtype=np.float32),
            "wmat": wm.astype(BF16),
            "iotp": iota_p.astype(BF16),
            "iotn": (-iota_p).astype(BF16),
            "iot4n": np.tile(-np.arange(d, dtype=np.float32), (d, 4)).astype(BF16),
            "pcol": np.arange(d, dtype=np.float32)[:, None].astype(BF16),
            "sidx": sidx_all[c],
            "rel": rel_all[c].astype(BF16),
        })
    return in_maps


def assemble_output(results, cfg: Cfg):
    outs = []
    for c in range(cfg.n_cores):
        dxT = np.asarray(results[c]["dxT"], np.float32)
        outs.append(dxT.T[:cfg.npc])
    return np.concatenate(outs, axis=0)


# -------------------------------------------------------- numpy device emulation

def _unwrap_idx(w, n):
    return np.ascontiguousarray(w[:16, :].T).reshape(-1)[:n]


def emulate(in_maps, cfg: Cfg):
    """Bit-approximate numpy emulation of the device program (fp32 accum)."""
    d = cfg.d
    NLO = cfg.g * cfg.lcpt * 128
    NHI = cfg.g * cfg.hcpt * 128
    results = []
    xg = np.asarray(in_maps[0]["xg"], np.float32)

    def gathered_block(table_f32, im, gi, blk):
        w = im["sidx"][gi]
        lo = _unwrap_idx(w[:, :NLO // 16], NLO).astype(np.int64)
        hi = _unwrap_idx(w[:, NLO // 16:], NHI).astype(np.int64) + cfg.hl
        allidx = np.concatenate([lo, hi])
        sel = allidx[blk * 128:(blk + 1) * 128]
        return table_f32[sel]

    def seg_psum(table_f32, im, gi, g):
        rel = np.asarray(im["rel"][gi], np.float32)
        psum = np.zeros((d, d), np.float32)
        for cc in range(cfg.cpt):
            blk = cfg.block_of(g, cc)
            rows = gathered_block(table_f32, im, gi, blk)
            rcol = rel[:, g * cfg.cpt + cc:g * cfg.cpt + cc + 1]
            ind = (-rcol == np.arange(d)[None, :]).astype(np.float32)
            psum += rows.T @ ind
        return psum

    g2loc_all, d1T_all = [], []
    for c in range(cfg.n_cores):
        im = in_maps[c]
        wm = np.asarray(im["wmat"], np.float32)
        d1T = np.zeros((d, cfg.npad), np.float32)
        g2loc = np.zeros((cfg.npad, d), np.float32)
        for ti in range(cfg.tiles):
            gi, g = divmod(ti, cfg.g)
            psum = seg_psum(xg, im, gi, g)
            xt_t = np.asarray(im["xt"][gi], np.float32)
            psum += wm[:, 0:d].T @ xt_t[:, (2 * g) * d:(2 * g + 1) * d]
            psum += wm[:, d:2 * d].T @ xt_t[:, (2 * g + 1) * d:(2 * g + 2) * d]
            psum += np.asarray(im["a1"][gi], np.float32)[:, g * d:(g + 1) * d]
            pbi = np.asarray(im["svi"], np.float32)[0, ti * d:(ti + 1) * d]
            d1t = (psum * pbi[None, :]).astype(BF16).astype(np.float32)
            d1T[:, ti * 128:(ti + 1) * 128] = d1t
            g2loc[ti * 128:(ti + 1) * 128] = (d1t.T @ wm[:, 2 * d:3 * d]).astype(
                BF16).astype(np.float32)
        g2loc_all.append(g2loc)
        d1T_all.append(d1T)
    GP = cfg.g * 128
    g2full = np.zeros((cfg.npf, d), np.float32)
    for gi in range(cfg.ng):
        for c in range(cfg.n_cores):
            g2full[(gi * cfg.n_cores + c) * GP:(gi * cfg.n_cores + c + 1) * GP] = \
                g2loc_all[c][gi * GP:(gi + 1) * GP]
    g2full = g2full.astype(BF16).astype(np.float32)
    for c in range(cfg.n_cores):
        im = in_maps[c]
        wm = np.asarray(im["wmat"], np.float32)
        d1T = d1T_all[c]
        dxT = np.zeros((d, cfg.npad), np.float32)
        for ti in range(cfg.tiles):
            gi, g = divmod(ti, cfg.g)
            psum = seg_psum(g2full, im, gi, g)
            svcv = np.asarray(im["svc"], np.float32)[0]
            d1t = d1T[:, ti * 128:(ti + 1) * 128]
            for kk in range(4):
                cn = svcv[ti * 512 + kk * d:ti * 512 + (kk + 1) * d]
                sc = (d1t * cn[None, :]).astype(BF16).astype(np.float32)
                psum += wm[:, (3 + kk) * d:(4 + kk) * d].T @ sc
            psum += np.asarray(im["a2"][gi], np.float32)[:, g * d:(g + 1) * d]
            pbi = np.asarray(im["svi"], np.float32)[0, ti * d:(ti + 1) * d]
            dxT[:, ti * 128:(ti + 1) * 128] = psum * pbi[None, :]
        results.append({"dxT": dxT})
    return results


# ----------------------------------------------------------------------- entry

_NC_CACHE = {}


def _needed_chunks(edge_index_int, cfg0: Cfg):
    """Per-(core,tile,half) 128-edge chunk ceilings for the actual edges."""
    src = np.asarray(edge_index_int[0], np.int64)
    tgt = np.asarray(edge_index_int[1], np.int64)
    core_of = tgt // cfg0.npc
    loc = tgt % cfg0.npc
    tile_of = loc // 128
    src_pad = (src // cfg0.npc) * cfg0.npad + (src % cfg0.npc)
    is_hi = (src_pad >= cfg0.hl).astype(np.int64)
    nb = cfg0.n_cores * cfg0.tiles
    lo_cnt = np.bincount((core_of * cfg0.tiles + tile_of)[is_hi == 0],
                         minlength=nb)
    hi_cnt = np.bincount((core_of * cfg0.tiles + tile_of)[is_hi == 1],
                         minlength=nb)
    need_l = max(1, int(np.ceil(lo_cnt.max() / 128)))
    need_h = max(1, int(np.ceil(hi_cnt.max() / 128)))
    return need_l, need_h


def kernel(**inputs) -> np.ndarray:
    cfg = Cfg()
    need_l, need_h = _needed_chunks(inputs["edge_index_int"], cfg)
    if need_l > cfg.lcpt or need_h > cfg.hcpt:
        cfg = Cfg(lcpt=max(need_l, cfg.lcpt), hcpt=max(need_h, cfg.hcpt))
    in_maps = host_prep(
        inputs["t"], inputs["x_int_t"], inputs["X_bound"], inputs["U"],
        inputs["edge_index_int"], inputs["edge_index_bound"],
        inputs["edge_index_ctrl"], inputs["timestamps"],
        inputs["params1"], inputs["params2"], cfg)
    key = (cfg.n_cores, cfg.npc, cfg.lcpt, cfg.hcpt, cfg.g)
    if key not in _NC_CACHE:
        _NC_CACHE[key] = build_nc(cfg)
    nc = _NC_CACHE[key]
    res = bass_utils.run_bass_kernel_spmd(
        nc, in_maps, core_ids=list(range(cfg.n_cores)))
    return assemble_output(res.results, cfg)
